# revision 31
# baseline (speedup 1.0000x reference)
"""Multi-head attention layer (N=4, L=S=2048, D=1024, H=16) on 8 TRN2 NeuronCores.

Sharding v3: 8 cores = 4 batches x 2 HEAD-HALVES (tensor parallel on heads).
Each core computes Q/K/V projections for its 8 heads (512 of 1024 dims) over
the FULL 2048 queries/keys of its batch, attention for those 8 heads, and a
PARTIAL output projection (contraction over its 512 dims). The host sums the
two partial outputs of each batch pair (unshard of the sum-sharded output).
vs the old (batch x query-half) sharding this halves the K/V projection
matmul work per core: 1536 512-col matmuls (~349us streaming) vs 1760.

Per-core data layout (host-prepared, bf16):
  xq/xk/xv [128, 8, 2048]  x[p,t,s] = input[n, s, t*128+p]   (transposed)
  wq/wk/wv [128, 8, 512]   w[p,t,d] = W[t*128+p, hh*512+d]
  wo [128, 4, 1024]        wo[p,t,d] = Wo[hh*512 + t*128+p, d]
  bq [128, 4] f32 (bk dropped: a per-query score shift is softmax-invariant)
  bv [64, 8] f32; bo [128, 1024] f32 pre-broadcast (zeros on the hh=1 core
  so the host sum adds bo exactly once)
  out [2048, 1024] f32 partial (natural layout)

Schedule: same PE-dense pending-queue design as v2 — attention units are
(dt=head-pair 0..3, lb=512-query-block 0..3); all projection groups ride as
filler inside the units; PV lags the exp stream; softmax denominator via the
ones-column of augmented V; O-projection groups gated per-lb on normalize
completion; final 8 O-groups use the early-accumulator trick so only their
last contraction step trails the last unit.
"""

import numpy as np
import ml_dtypes

import concourse.bass as bass
import concourse.mybir as mybir
import concourse.tile as tile
from concourse import bacc
from concourse.bass_utils import run_bass_kernel_spmd

BF16 = mybir.dt.bfloat16
F32 = mybir.dt.float32
ALU = mybir.AluOpType
ACTF = mybir.ActivationFunctionType

N, L, S, D, H, E = 4, 2048, 2048, 1024, 16, 64
DH = 512                 # dims per core = 8 heads
N_CORES = 8

_nc_cache = None
last_results = None


def _build():
    nc = bacc.Bacc(None, target_bir_lowering=False)

    xq = nc.declare_dram_parameter("xq", [128, 8, L], BF16, isOutput=False)
    xk = nc.declare_dram_parameter("xk", [128, 8, S], BF16, isOutput=False)
    xv = nc.declare_dram_parameter("xv", [128, 8, S], BF16, isOutput=False)
    wq = nc.declare_dram_parameter("wq", [128, 8, DH], BF16, isOutput=False)
    wk = nc.declare_dram_parameter("wk", [128, 8, DH], BF16, isOutput=False)
    wv = nc.declare_dram_parameter("wv", [128, 8, DH], BF16, isOutput=False)
    wo = nc.declare_dram_parameter("wo", [128, 4, D], BF16, isOutput=False)
    bq = nc.declare_dram_parameter("bq", [128, 4], F32, isOutput=False)
    bv = nc.declare_dram_parameter("bv", [64, 8], F32, isOutput=False)
    bo = nc.declare_dram_parameter("bo", [128, D], F32, isOutput=False)
    out = nc.declare_dram_parameter("out", [L, D], F32, isOutput=True)

    with tile.TileContext(nc) as tc:
        with tc.tile_pool(name="const", bufs=1) as cpool, \
             tc.tile_pool(name="pers", bufs=1) as ppool, \
             tc.tile_pool(name="stage", bufs=3) as spool, \
             tc.tile_pool(name="vstage", bufs=2) as vpool, \
             tc.tile_pool(name="qstage", bufs=2) as qpool, \
             tc.tile_pool(name="work", bufs=2) as wpool, \
             tc.tile_pool(name="expp", bufs=7) as epool, \
             tc.tile_pool(name="psum", bufs=2, space="PSUM") as psum:

            wq_t = cpool.tile([128, 8, DH], BF16, tag="w_q")
            wk_t = cpool.tile([128, 8, DH], BF16, tag="w_k")
            wv_t = cpool.tile([128, 8, DH], BF16, tag="w_v")
            wo_t = cpool.tile([128, 4, D], BF16, tag="w_o")
            bq_t = cpool.tile([128, 4], F32, tag="bq")
            bv_t = cpool.tile([64, 8], F32, tag="bv")
            bo_t = cpool.tile([128, D], F32, tag="bo")
            qT = ppool.tile([128, 4, L], BF16, tag="qT")
            kT = ppool.tile([128, 4, S], BF16, tag="kT")
            vaug = ppool.tile([128, 16, 8 * 65], BF16, tag="vaug")
            oT = ppool.tile([128, 4, L], BF16, tag="oT")

            # ---- critical-path DMAs: K(0,0) + Q(0,0) inputs first ----
            # ct0 slivers first so the first matmul can start ~1us in
            nc.sync.dma_start(wk_t[:, 0:1, 0:128], wk[:, 0:1, 0:128])
            sgk0 = spool.tile([128, 8, 512], BF16, tag="stage")
            nc.sync.dma_start(sgk0[:, 0:1, :], xk[:, 0:1, 0:512])
            nc.sync.dma_start(wk_t[:, 1:8, 0:128], wk[:, 1:8, 0:128])
            nc.sync.dma_start(sgk0[:, 1:8, :], xk[:, 1:8, 0:512])
            nc.sync.dma_start(wq_t[:, :, 0:128], wq[:, :, 0:128])
            sgq0 = qpool.tile([128, 8, 512], BF16, tag="qstage")
            nc.sync.dma_start(sgq0[:], xq[:, :, 0:512])
            nc.sync.dma_start(bq_t[:], bq[:])

            # warm the exp table-set while DMAs fill (one tiny ACTIVATE)
            wrm = wpool.tile([1, 16], F32, tag="warm")
            nc.vector.memset(wrm[:], 0.0)
            wrm2 = wpool.tile([1, 16], F32, tag="warm2")
            nc.scalar.activation(wrm2[:], wrm[:], ACTF.Exp, scale=0.125)

            # ones column (slot 64) of augmented V => PV row 64 = softmax denom
            for st in range(16):
                v3 = vaug[:, st].rearrange("p (h e) -> p h e", e=65)
                nc.vector.memset(v3[:, :, 64:65], 1.0)

            # ---- projection-group emitters ----
            def proj_group(w_t, sg_t, dt, dst, bias, on_act=False):
                ps = psum.tile([128, 512], F32, tag="mm512", bufs=2)
                for ct in range(8):
                    nc.tensor.matmul(ps[:], w_t[:, ct, dt * 128:(dt + 1) * 128],
                                     sg_t[:, ct, :], start=(ct == 0),
                                     stop=(ct == 7))
                # evacuate on the scalar engine while it still has idle time
                # (era0) so the DVE FIFO doesn't delay mm512 psum recycling
                if on_act:
                    nc.scalar.activation(dst, ps[:], ACTF.Identity,
                                         bias=0.0 if bias is None else bias)
                elif bias is None:
                    nc.vector.tensor_copy(dst, ps[:])
                else:
                    nc.vector.tensor_scalar_add(dst, ps[:], bias)

            def v_proj_group(sg_t, st):
                stl = st % 4
                ps = psum.tile([128, 512], F32, tag="mm512", bufs=2)
                for ct in range(8):
                    nc.tensor.matmul(ps[:], sg_t[:, ct, stl * 128:(stl + 1) * 128],
                                     wv_t[:, ct, :], start=(ct == 0),
                                     stop=(ct == 7))
                v3 = vaug[:, st].rearrange("p (h e) -> p h e", e=65)
                # evacuate on the scalar engine (idle during era0) so the DVE
                # FIFO doesn't delay mm512 psum recycling for later groups
                nc.scalar.activation(
                    v3[:, :, 0:64],
                    ps[:].rearrange("p (h e) -> p h e", e=64),
                    ACTF.Identity)

            def o_proj_group(lt, db):
                ps = psum.tile([128, 512], F32, tag="mm512", bufs=2)
                for ct in range(4):
                    nc.tensor.matmul(ps[:], oT[:, ct, lt * 128:(lt + 1) * 128],
                                     wo_t[:, ct, db * 512:(db + 1) * 512],
                                     start=(ct == 0), stop=(ct == 3))
                ob = wpool.tile([128, 512], F32, tag="outsb")
                nc.vector.tensor_add(ob[:], ps[:],
                                     bo_t[:, db * 512:(db + 1) * 512])
                nc.sync.dma_start(
                    out[lt * 128:(lt + 1) * 128, db * 512:(db + 1) * 512], ob[:])

            # ---- feed machinery ----
            emitted = set()        # keys of emitted groups
            v_ready = [0]          # count of emitted V st-groups
            norm_done = [0, 0, 0, 0]   # per lb: fully-normalized units

            k_box, q_box, v_box = [None], [None], [None]

            def k_item(dt, sb, fresh=False):
                def dma():
                    if fresh:
                        sg = spool.tile([128, 8, 512], BF16, tag="stage")
                        nc.sync.dma_start(sg[:],
                                          xk[:, :, sb * 512:(sb + 1) * 512])
                        k_box[0] = sg
                    return k_box[0]
                def compute(sg):
                    proj_group(wk_t, sg, dt, kT[:, dt, sb * 512:(sb + 1) * 512],
                               None, on_act=True)
                    emitted.add(("k", dt, sb))
                return (dma, compute, ("k", dt, sb))

            def q_item(dt, lb, fresh=False):
                def dma():
                    if fresh:
                        sg = qpool.tile([128, 8, 512], BF16, tag="qstage")
                        nc.sync.dma_start(sg[:],
                                          xq[:, :, lb * 512:(lb + 1) * 512])
                        q_box[0] = sg
                    return q_box[0]
                def compute(sg):
                    proj_group(wq_t, sg, dt, qT[:, dt, lb * 512:(lb + 1) * 512],
                               bq_t[:, dt:dt + 1], on_act=(lb <= 1))
                    emitted.add(("q", dt, lb))
                return (dma, compute, ("q", dt, lb))

            def v_item(st, fresh=False):
                sb = st // 4
                def dma():
                    if fresh:
                        sg = vpool.tile([128, 8, 512], BF16, tag="vstage")
                        nc.sync.dma_start(sg[:],
                                          xv[:, :, sb * 512:(sb + 1) * 512])
                        v_box[0] = sg
                    return v_box[0]
                def compute(sg):
                    v_proj_group(sg, st)
                    v_ready[0] += 1
                    emitted.add(("v", st, 0))
                return (dma, compute, ("v", st, 0))

            def dma_feed(fn, name):
                return (None, lambda sg: fn(), ("w", name, 0))

            def o_item(lt, db):
                def compute(sg):
                    o_proj_group(lt, db)
                    emitted.add(("o", lt, db))
                return (None, compute, ("o", lt, db))

            # ---- slot-scheduled feed: emit each filler group near its
            # need-slot so PE filler spreads across all 256 unit st-slots
            # (a clustered feed leaves later eras exp-latency-bound).
            # admit (=issue input DMA) LEAD slots before the emit slot.
            LEAD = 5
            admit_sched = {}       # slot -> [item]
            emit_slot = {}         # key -> scheduled emit slot
            slot_ctr = [0]
            pending = []           # [(compute, sg, key)] admitted, not emitted
            deferred = []          # admitted but gated at emit time

            def at(sl, it):
                admit_sched.setdefault(max(0, sl - LEAD), []).append(it)
                emit_slot[it[2]] = sl

            def item_ok(it):
                key = it[2]
                return key[0] != "o" or norm_done[key[1] // 4] >= 4

            def emit(entry):
                compute, sg, key = entry
                compute(sg)

            def admit(it):
                dma, compute, key = it
                sg = dma() if dma else None
                pending.append((compute, sg, key))

            def pump():
                sl = slot_ctr[0]
                slot_ctr[0] += 1
                for it in admit_sched.pop(sl, []):
                    admit(it)
                for entry in list(deferred):
                    if item_ok((None, None, entry[2])):
                        deferred.remove(entry)
                        emit(entry)
                while pending and emit_slot.get(pending[0][2], 0) <= sl:
                    entry = pending.pop(0)
                    if item_ok((None, None, entry[2])):
                        emit(entry)
                    else:
                        deferred.append(entry)

            def ensure(kind, a, b):
                key = (kind, a, b)
                guard = 0
                while key not in emitted:
                    guard += 1
                    if guard > 10000:
                        raise RuntimeError(f"cannot satisfy prereq {key}")
                    # force-drain: admit everything scheduled so far, emit
                    # pending in order until the prereq lands
                    if pending:
                        emit(pending.pop(0))
                    elif admit_sched:
                        sl = min(admit_sched)
                        for it in admit_sched.pop(sl):
                            admit(it)
                    else:
                        raise RuntimeError(f"cannot satisfy prereq {key}")

            # schedule: V(st) ~st; K(dt,sb) before unit(dt,0) consumes it;
            # Q(dt,lb) ~8 slots before unit(dt,lb); O(lb) spread over the 64
            # slots after lb completes; wo/bo mid-era0.
            for st in range(1, 16):
                at(st, v_item(st, fresh=(st % 4 == 0)))
            for dt in range(1, 4):
                for sb in range(4):
                    at(16 * dt + 4 * sb - 6, k_item(dt, sb, fresh=True))
            for lb in range(4):
                for dt in range(4):
                    if lb == 0 and dt == 0:
                        continue
                    sl = 64 * lb + 16 * dt - 8
                    at(sl, q_item(dt, lb, fresh=(lb > 0 and dt == 0)))
            at(40, dma_feed(lambda: nc.sync.dma_start(wo_t[:], wo[:]), "wo"))
            at(44, dma_feed(lambda: nc.sync.dma_start(bo_t[:], bo[:]), "bo"))
            for lb in range(3):
                for i, (lt, db) in enumerate(
                        (lt, db) for lt in range(4 * lb, 4 * lb + 4)
                        for db in range(2)):
                    at(64 * (lb + 1) + 2 + 7 * i, o_item(lt, db))

            # ---- attention units with lagging PV ----
            units = []

            def drain_pv():
                while units:
                    u = units[0]
                    # keep PV two steps behind the exp stream while the
                    # unit's scores are still running, so PV matmuls never
                    # reach the engine before their exp has completed
                    lim = u["exp_n"] if u.get("done") else u["exp_n"] - 2
                    while (u["next"] < 16 and u["next"] < v_ready[0]
                           and u["next"] < lim):
                        st = u["next"]
                        if st == 0:
                            u["pe"] = psum.tile([128, 512], F32, tag="pepo",
                                                bufs=2, name="pe_acc")
                            u["po"] = psum.tile([128, 512], F32, tag="pepo",
                                                bufs=2, name="po_acc")
                        he, ho = 2 * u["dt"], 2 * u["dt"] + 1
                        ep = u["ep"][st]
                        nc.tensor.matmul(u["pe"][0:65, :],
                                         vaug[:, st, he * 65:(he + 1) * 65],
                                         ep[:, 0:512],
                                         start=(st == 0), stop=(st == 15))
                        nc.tensor.matmul(u["po"][0:65, :],
                                         vaug[:, st, ho * 65:(ho + 1) * 65],
                                         ep[:, 512:1024],
                                         start=(st == 0), stop=(st == 15))
                        u["ep"][st] = None
                        u["next"] += 1
                    if u["next"] == 16:
                        unit_epilogue(u)
                        units.pop(0)
                    else:
                        break

            def normalize(cp, h, lb):
                # cp: [65, 512] f32 SBUF; row 64 = softmax denominator
                den0 = wpool.tile([1, 512], F32, tag="den0")
                nc.sync.dma_start(den0[0:1, :], cp[64:65, :])
                recb = wpool.tile([64, 512], F32, tag="recb")
                nc.gpsimd.partition_broadcast(recb[:], den0[0:1, :])
                nc.vector.reciprocal_approx_fast(recb[:], recb[:])
                dt = h // 2
                if h % 2 == 0:
                    dst = oT[0:64, dt, lb * 512:(lb + 1) * 512]
                    nc.vector.tensor_tensor(dst, cp[0:64, :], recb[:], ALU.mult)
                    nc.vector.tensor_scalar_add(dst, dst, bv_t[:, h:h + 1])
                else:
                    tmp = wpool.tile([64, 512], BF16, tag="otmp")
                    nc.vector.tensor_tensor(tmp[:], cp[0:64, :], recb[:],
                                            ALU.mult)
                    nc.vector.tensor_scalar_add(tmp[:], tmp[:], bv_t[:, h:h + 1])
                    nc.sync.dma_start(
                        oT[64:128, dt, lb * 512:(lb + 1) * 512], tmp[:])

            def unit_epilogue(u):
                dt, lb = u["dt"], u["lb"]
                cpe = wpool.tile([65, 512], F32, tag="cpe")
                nc.vector.tensor_copy(cpe[:], u["pe"][0:65, :])
                cpo = wpool.tile([65, 512], F32, tag="cpo")
                nc.vector.tensor_copy(cpo[:], u["po"][0:65, :])
                normalize(cpe, 2 * dt, lb)
                normalize(cpo, 2 * dt + 1, lb)
                norm_done[lb] += 1

            def attention_unit(dt, lb):
                ensure("q", dt, lb)
                ensure("k", dt, 0)
                u = {"dt": dt, "lb": lb, "next": 0, "exp_n": 0,
                     "ep": [None] * 16}
                units.append(u)
                qe = qT[0:64, dt, lb * 512:(lb + 1) * 512]
                qo = qT[64:128, dt, lb * 512:(lb + 1) * 512]
                for st in range(16):
                    if st % 4 == 0 and st > 0:
                        ensure("k", dt, st // 4)
                    ps2 = psum.tile([128, 1024], F32, tag="sc2", bufs=2)
                    nc.tensor.matmul(ps2[:, 0:512],
                                     kT[0:64, dt, st * 128:(st + 1) * 128],
                                     qe, start=True, stop=True)
                    nc.tensor.matmul(ps2[:, 512:1024],
                                     kT[64:128, dt, st * 128:(st + 1) * 128],
                                     qo, start=True, stop=True)
                    ep = epool.tile([128, 1024], BF16, tag="ep")
                    nc.scalar.activation(ep[:], ps2[:], ACTF.Exp, scale=0.125)
                    u["ep"][st] = ep
                    u["exp_n"] = st + 1
                    pump()
                    drain_pv()
                u["done"] = True

            # ---- startup: K(0,0)/Q(0,0) immediately, then prefetch ----
            kd, kc, _ = k_item(0, 0)
            kc(sgk0)
            qd, qc, _ = q_item(0, 0)
            qc(sgq0)
            q_box[0] = sgq0    # Q(1..3, 0) reuse the startup xq chunk

            # DMA issue order = need order: xk1 for K(0,1), wv + xv0 for the
            # first V groups, then xk2/xk3 and remaining weights
            it = k_item(0, 1, fresh=True)
            admit(it)
            emit_slot[it[2]] = 2
            nc.sync.dma_start(wv_t[:], wv[:])
            it = v_item(0, fresh=True)
            admit(it)
            emit_slot[it[2]] = 1
            it = k_item(0, 2, fresh=True)
            admit(it)
            emit_slot[it[2]] = 5
            it = k_item(0, 3, fresh=True)
            admit(it)
            emit_slot[it[2]] = 9
            nc.sync.dma_start(wq_t[:, :, 128:512], wq[:, :, 128:512])
            nc.sync.dma_start(wk_t[:, :, 128:512], wk[:, :, 128:512])
            nc.sync.dma_start(bv_t[:], bv[:])

            # ---- attention sweeps ----
            for lb in range(4):
                for dt in range(4):
                    attention_unit(dt, lb)

            # ---- flush: finish lagging PVs, then any remaining filler ----
            for _ in range(4096):
                drain_pv()
                stepped = False
                for entry in list(deferred):
                    if item_ok((None, None, entry[2])):
                        deferred.remove(entry)
                        emit(entry)
                        stepped = True
                        break
                if not stepped and pending:
                    entry = pending.pop(0)
                    if item_ok((None, None, entry[2])):
                        emit(entry)
                    else:
                        deferred.append(entry)
                    stepped = True
                if not stepped and admit_sched:
                    sl = min(admit_sched)
                    for it in admit_sched.pop(sl):
                        admit(it)
                    stepped = True
                if not stepped and not units:
                    break
            assert not units and not pending and not deferred and \
                not admit_sched

            # final O-proj groups (lt 12..15): six accumulators run their
            # ct0-2 matmuls during the last unit's PV/normalize; only ct3 +
            # epilogue (and two full groups) remain after it.
            finals = [(lt, db) for lt in range(12, 16) for db in range(2)]
            accs = []
            for i, (lt, db) in enumerate(finals[:6]):
                if i < 2:
                    ps = psum.tile([128, 512], F32, tag="mm512", bufs=2,
                                   name="oaccA")
                    ap = ps[:]
                elif i < 4:
                    ps = psum.tile([128, 1024], F32, tag="sc2", bufs=2,
                                   name="oaccB")
                    ap = ps[:, 0:512]
                else:
                    ps = psum.tile([128, 512], F32, tag="pepo", bufs=2,
                                   name="oaccC")
                    ap = ps[:]
                for ct in range(3):
                    nc.tensor.matmul(
                        ap, oT[:, ct, lt * 128:(lt + 1) * 128],
                        wo_t[:, ct, db * 512:(db + 1) * 512],
                        start=(ct == 0), stop=False)
                accs.append(ap)
            for (lt, db), ap in zip(finals[:6], accs):
                nc.tensor.matmul(
                    ap, oT[:, 3, lt * 128:(lt + 1) * 128],
                    wo_t[:, 3, db * 512:(db + 1) * 512],
                    start=False, stop=True)
                ob = wpool.tile([128, 512], F32, tag="outsb")
                nc.vector.tensor_add(ob[:], ap,
                                     bo_t[:, db * 512:(db + 1) * 512])
                nc.sync.dma_start(
                    out[lt * 128:(lt + 1) * 128,
                        db * 512:(db + 1) * 512], ob[:])
            for lt, db in finals[6:]:
                o_proj_group(lt, db)

    nc.compile()
    return nc


def _pack_kxm(w):
    k, m = w.shape
    return np.ascontiguousarray(
        w.reshape(k // 128, 128, m).transpose(1, 0, 2)).astype(ml_dtypes.bfloat16)


def kernel(queries, keys, values, Wq, bq, Wk, bk, Wv, bv, Wo, bo):
    global _nc_cache, last_results
    queries = np.asarray(queries, dtype=np.float32)
    keys = np.asarray(keys, dtype=np.float32)
    values = np.asarray(values, dtype=np.float32)

    if _nc_cache is None:
        _nc_cache = _build()
    nc = _nc_cache

    Wq = np.asarray(Wq, np.float32)
    Wk = np.asarray(Wk, np.float32)
    Wv = np.asarray(Wv, np.float32)
    Wo = np.asarray(Wo, np.float32)
    bq = np.asarray(bq, np.float32)
    bv = np.asarray(bv, np.float32)
    bo = np.asarray(bo, np.float32)

    # per head-half hh: weight slices + biases (bk dropped: softmax-invariant)
    half = {}
    for hh in range(2):
        sl = slice(hh * DH, (hh + 1) * DH)
        half[hh] = {
            "wq": _pack_kxm(Wq[:, sl]),
            "wk": _pack_kxm(Wk[:, sl]),
            "wv": _pack_kxm(Wv[:, sl]),
            "wo": _pack_kxm(Wo[sl, :]),
            "bq": np.ascontiguousarray(bq[sl].reshape(4, 128).T),
            "bv": np.ascontiguousarray(bv[sl].reshape(8, 64).T),
            "bo": np.ascontiguousarray(
                np.broadcast_to(bo, (128, D)) if hh == 0
                else np.zeros((128, D), np.float32)),
        }

    xs = {}
    for n in range(N):
        xs[n] = {
            "xq": _pack_kxm(np.ascontiguousarray(queries[n].T)),
            "xk": _pack_kxm(np.ascontiguousarray(keys[n].T)),
            "xv": _pack_kxm(np.ascontiguousarray(values[n].T)),
        }

    in_maps = []
    for c in range(N_CORES):
        n, hh = c // 2, c % 2
        m = dict(half[hh])
        m.update(xs[n])
        in_maps.append(m)

    last_results = run_bass_kernel_spmd(nc, in_maps, list(range(N_CORES)))

    full = np.empty((N, L, D), np.float32)
    for n in range(N):
        full[n] = last_results.results[2 * n]["out"]
        full[n] += last_results.results[2 * n + 1]["out"]
    return full


# revision 32
# speedup vs baseline: 1.0151x; 1.0151x over previous
"""Multi-head attention layer (N=4, L=S=2048, D=1024, H=16) on 8 TRN2 NeuronCores.

Sharding v3: 8 cores = 4 batches x 2 HEAD-HALVES (tensor parallel on heads).
Each core computes Q/K/V projections for its 8 heads (512 of 1024 dims) over
the FULL 2048 queries/keys of its batch, attention for those 8 heads, and a
PARTIAL output projection (contraction over its 512 dims). The host sums the
two partial outputs of each batch pair (unshard of the sum-sharded output).
vs the old (batch x query-half) sharding this halves the K/V projection
matmul work per core: 1536 512-col matmuls (~349us streaming) vs 1760.

Per-core data layout (host-prepared, bf16):
  xq/xk/xv [128, 8, 2048]  x[p,t,s] = input[n, s, t*128+p]   (transposed)
  wq/wk/wv [128, 8, 512]   w[p,t,d] = W[t*128+p, hh*512+d]
  wo [128, 4, 1024]        wo[p,t,d] = Wo[hh*512 + t*128+p, d]
  bq [128, 4] f32 (bk dropped: a per-query score shift is softmax-invariant)
  bv [64, 8] f32; bo [128, 1024] f32 pre-broadcast (zeros on the hh=1 core
  so the host sum adds bo exactly once)
  out [2048, 1024] f32 partial (natural layout)

Schedule: same PE-dense pending-queue design as v2 — attention units are
(dt=head-pair 0..3, lb=512-query-block 0..3); all projection groups ride as
filler inside the units; PV lags the exp stream; softmax denominator via the
ones-column of augmented V; O-projection groups gated per-lb on normalize
completion; final 8 O-groups use the early-accumulator trick so only their
last contraction step trails the last unit.
"""

import numpy as np
import ml_dtypes

import concourse.bass as bass
import concourse.mybir as mybir
import concourse.tile as tile
from concourse import bacc
from concourse.bass_utils import run_bass_kernel_spmd

BF16 = mybir.dt.bfloat16
F32 = mybir.dt.float32
ALU = mybir.AluOpType
ACTF = mybir.ActivationFunctionType

N, L, S, D, H, E = 4, 2048, 2048, 1024, 16, 64
DH = 512                 # dims per core = 8 heads
N_CORES = 8

_nc_cache = None
last_results = None


def _build():
    nc = bacc.Bacc(None, target_bir_lowering=False)

    xq = nc.declare_dram_parameter("xq", [128, 8, L], BF16, isOutput=False)
    xk = nc.declare_dram_parameter("xk", [128, 8, S], BF16, isOutput=False)
    xv = nc.declare_dram_parameter("xv", [128, 8, S], BF16, isOutput=False)
    wq = nc.declare_dram_parameter("wq", [128, 8, DH], BF16, isOutput=False)
    wk = nc.declare_dram_parameter("wk", [128, 8, DH], BF16, isOutput=False)
    wv = nc.declare_dram_parameter("wv", [128, 8, DH], BF16, isOutput=False)
    wo = nc.declare_dram_parameter("wo", [128, 4, D], BF16, isOutput=False)
    bq = nc.declare_dram_parameter("bq", [128, 4], F32, isOutput=False)
    bv = nc.declare_dram_parameter("bv", [64, 8], F32, isOutput=False)
    bo = nc.declare_dram_parameter("bo", [128, D], F32, isOutput=False)
    out = nc.declare_dram_parameter("out", [L, D], F32, isOutput=True)

    with tile.TileContext(nc) as tc:
        with tc.tile_pool(name="const", bufs=1) as cpool, \
             tc.tile_pool(name="pers", bufs=1) as ppool, \
             tc.tile_pool(name="stage", bufs=3) as spool, \
             tc.tile_pool(name="vstage", bufs=2) as vpool, \
             tc.tile_pool(name="qstage", bufs=2) as qpool, \
             tc.tile_pool(name="work", bufs=2) as wpool, \
             tc.tile_pool(name="expp", bufs=7) as epool, \
             tc.tile_pool(name="psum", bufs=2, space="PSUM") as psum:

            wq_t = cpool.tile([128, 8, DH], BF16, tag="w_q")
            wk_t = cpool.tile([128, 8, DH], BF16, tag="w_k")
            wv_t = cpool.tile([128, 8, DH], BF16, tag="w_v")
            wo_t = cpool.tile([128, 4, D], BF16, tag="w_o")
            bq_t = cpool.tile([128, 4], F32, tag="bq")
            bv_t = cpool.tile([64, 8], F32, tag="bv")
            bo_t = cpool.tile([128, D], F32, tag="bo")
            qT = ppool.tile([128, 4, L], BF16, tag="qT")
            kT = ppool.tile([128, 4, S], BF16, tag="kT")
            vaug = ppool.tile([128, 16, 8 * 65], BF16, tag="vaug")
            oT = ppool.tile([128, 4, L], BF16, tag="oT")

            # ---- critical-path DMAs: K(0,0) + Q(0,0) inputs first ----
            # ct0 slivers first so the first matmul can start ~1us in
            nc.sync.dma_start(wk_t[:, 0:1, 0:128], wk[:, 0:1, 0:128])
            sgk0 = spool.tile([128, 8, 512], BF16, tag="stage")
            nc.sync.dma_start(sgk0[:, 0:1, :], xk[:, 0:1, 0:512])
            nc.sync.dma_start(wk_t[:, 1:8, 0:128], wk[:, 1:8, 0:128])
            nc.sync.dma_start(sgk0[:, 1:8, :], xk[:, 1:8, 0:512])
            nc.sync.dma_start(wq_t[:, :, 0:128], wq[:, :, 0:128])
            sgq0 = qpool.tile([128, 8, 512], BF16, tag="qstage")
            nc.sync.dma_start(sgq0[:], xq[:, :, 0:512])
            nc.sync.dma_start(bq_t[:], bq[:])

            # warm the exp table-set while DMAs fill (one tiny ACTIVATE)
            wrm = wpool.tile([1, 16], F32, tag="warm")
            nc.vector.memset(wrm[:], 0.0)
            wrm2 = wpool.tile([1, 16], F32, tag="warm2")
            nc.scalar.activation(wrm2[:], wrm[:], ACTF.Exp, scale=0.125)

            # ones column (slot 64) of augmented V => PV row 64 = softmax denom
            for st in range(16):
                v3 = vaug[:, st].rearrange("p (h e) -> p h e", e=65)
                nc.vector.memset(v3[:, :, 64:65], 1.0)

            # ---- projection-group emitters ----
            def proj_group(w_t, sg_t, dt, dst, bias, on_act=False):
                ps = psum.tile([128, 512], F32, tag="mm512", bufs=2)
                for ct in range(8):
                    nc.tensor.matmul(ps[:], w_t[:, ct, dt * 128:(dt + 1) * 128],
                                     sg_t[:, ct, :], start=(ct == 0),
                                     stop=(ct == 7))
                # evacuate on the scalar engine while it still has idle time
                # (era0) so the DVE FIFO doesn't delay mm512 psum recycling
                if on_act:
                    nc.scalar.activation(dst, ps[:], ACTF.Identity,
                                         bias=0.0 if bias is None else bias)
                elif bias is None:
                    nc.vector.tensor_copy(dst, ps[:])
                else:
                    nc.vector.tensor_scalar_add(dst, ps[:], bias)

            def v_proj_group(sg_t, st):
                stl = st % 4
                ps = psum.tile([128, 512], F32, tag="mm512", bufs=2)
                for ct in range(8):
                    nc.tensor.matmul(ps[:], sg_t[:, ct, stl * 128:(stl + 1) * 128],
                                     wv_t[:, ct, :], start=(ct == 0),
                                     stop=(ct == 7))
                v3 = vaug[:, st].rearrange("p (h e) -> p h e", e=65)
                # evacuate on the scalar engine (idle during era0) so the DVE
                # FIFO doesn't delay mm512 psum recycling for later groups
                nc.scalar.activation(
                    v3[:, :, 0:64],
                    ps[:].rearrange("p (h e) -> p h e", e=64),
                    ACTF.Identity)

            def o_proj_group(lt, db):
                ps = psum.tile([128, 512], F32, tag="mm512", bufs=2)
                for ct in range(4):
                    nc.tensor.matmul(ps[:], oT[:, ct, lt * 128:(lt + 1) * 128],
                                     wo_t[:, ct, db * 512:(db + 1) * 512],
                                     start=(ct == 0), stop=(ct == 3))
                ob = wpool.tile([128, 512], F32, tag="outsb")
                nc.vector.tensor_add(ob[:], ps[:],
                                     bo_t[:, db * 512:(db + 1) * 512])
                nc.sync.dma_start(
                    out[lt * 128:(lt + 1) * 128, db * 512:(db + 1) * 512], ob[:])

            # ---- feed machinery ----
            emitted = set()        # keys of emitted groups
            v_ready = [0]          # count of emitted V st-groups
            norm_done = [0, 0, 0, 0]   # per lb: fully-normalized units

            k_box, q_box, v_box = [None], [None], [None]

            def k_item(dt, sb, fresh=False):
                def dma():
                    if fresh:
                        sg = spool.tile([128, 8, 512], BF16, tag="stage")
                        nc.sync.dma_start(sg[:],
                                          xk[:, :, sb * 512:(sb + 1) * 512])
                        k_box[0] = sg
                    return k_box[0]
                def compute(sg):
                    proj_group(wk_t, sg, dt, kT[:, dt, sb * 512:(sb + 1) * 512],
                               None)
                    emitted.add(("k", dt, sb))
                return (dma, compute, ("k", dt, sb))

            def q_item(dt, lb, fresh=False):
                def dma():
                    if fresh:
                        sg = qpool.tile([128, 8, 512], BF16, tag="qstage")
                        nc.sync.dma_start(sg[:],
                                          xq[:, :, lb * 512:(lb + 1) * 512])
                        q_box[0] = sg
                    return q_box[0]
                def compute(sg):
                    proj_group(wq_t, sg, dt, qT[:, dt, lb * 512:(lb + 1) * 512],
                               bq_t[:, dt:dt + 1])
                    emitted.add(("q", dt, lb))
                return (dma, compute, ("q", dt, lb))

            def v_item(st, fresh=False):
                sb = st // 4
                def dma():
                    if fresh:
                        sg = vpool.tile([128, 8, 512], BF16, tag="vstage")
                        nc.sync.dma_start(sg[:],
                                          xv[:, :, sb * 512:(sb + 1) * 512])
                        v_box[0] = sg
                    return v_box[0]
                def compute(sg):
                    v_proj_group(sg, st)
                    v_ready[0] += 1
                    emitted.add(("v", st, 0))
                return (dma, compute, ("v", st, 0))

            def dma_feed(fn, name):
                return (None, lambda sg: fn(), ("w", name, 0))

            def o_item(lt, db):
                def compute(sg):
                    o_proj_group(lt, db)
                    emitted.add(("o", lt, db))
                return (None, compute, ("o", lt, db))

            # ---- slot-scheduled feed: emit each filler group near its
            # need-slot so PE filler spreads across all 256 unit st-slots
            # (a clustered feed leaves later eras exp-latency-bound).
            # admit (=issue input DMA) LEAD slots before the emit slot.
            LEAD = 5
            admit_sched = {}       # slot -> [item]
            emit_slot = {}         # key -> scheduled emit slot
            slot_ctr = [0]
            pending = []           # [(compute, sg, key)] admitted, not emitted
            deferred = []          # admitted but gated at emit time

            def at(sl, it):
                admit_sched.setdefault(max(0, sl - LEAD), []).append(it)
                emit_slot[it[2]] = sl

            def item_ok(it):
                key = it[2]
                return key[0] != "o" or norm_done[key[1] // 4] >= 4

            def emit(entry):
                compute, sg, key = entry
                compute(sg)

            def admit(it):
                dma, compute, key = it
                sg = dma() if dma else None
                pending.append((compute, sg, key))

            def pump():
                sl = slot_ctr[0]
                slot_ctr[0] += 1
                for it in admit_sched.pop(sl, []):
                    admit(it)
                for entry in list(deferred):
                    if item_ok((None, None, entry[2])):
                        deferred.remove(entry)
                        emit(entry)
                while pending and emit_slot.get(pending[0][2], 0) <= sl:
                    entry = pending.pop(0)
                    if item_ok((None, None, entry[2])):
                        emit(entry)
                    else:
                        deferred.append(entry)

            def ensure(kind, a, b):
                key = (kind, a, b)
                guard = 0
                while key not in emitted:
                    guard += 1
                    if guard > 10000:
                        raise RuntimeError(f"cannot satisfy prereq {key}")
                    # force-drain: admit everything scheduled so far, emit
                    # pending in order until the prereq lands
                    if pending:
                        emit(pending.pop(0))
                    elif admit_sched:
                        sl = min(admit_sched)
                        for it in admit_sched.pop(sl):
                            admit(it)
                    else:
                        raise RuntimeError(f"cannot satisfy prereq {key}")

            # schedule: V(st) ~st; K(dt,sb) before unit(dt,0) consumes it;
            # Q(dt,lb) ~8 slots before unit(dt,lb); O(lb) spread over the 64
            # slots after lb completes; wo/bo mid-era0.
            for st in range(1, 16):
                at(st, v_item(st, fresh=(st % 4 == 0)))
            for dt in range(1, 4):
                for sb in range(4):
                    at(16 * dt + 4 * sb - 6, k_item(dt, sb, fresh=True))
            for lb in range(4):
                for dt in range(4):
                    if lb == 0 and dt == 0:
                        continue
                    sl = 64 * lb + 16 * dt - 8
                    at(sl, q_item(dt, lb, fresh=(lb > 0 and dt == 0)))
            at(40, dma_feed(lambda: nc.sync.dma_start(wo_t[:], wo[:]), "wo"))
            at(44, dma_feed(lambda: nc.sync.dma_start(bo_t[:], bo[:]), "bo"))
            for lb in range(3):
                for i, (lt, db) in enumerate(
                        (lt, db) for lt in range(4 * lb, 4 * lb + 4)
                        for db in range(2)):
                    at(64 * (lb + 1) + 2 + 7 * i, o_item(lt, db))

            # ---- attention units with lagging PV ----
            units = []

            def drain_pv():
                while units:
                    u = units[0]
                    # keep PV two steps behind the exp stream while the
                    # unit's scores are still running, so PV matmuls never
                    # reach the engine before their exp has completed
                    lim = u["exp_n"] if u.get("done") else u["exp_n"] - 2
                    while (u["next"] < 16 and u["next"] < v_ready[0]
                           and u["next"] < lim):
                        st = u["next"]
                        if st == 0:
                            u["pe"] = psum.tile([128, 512], F32, tag="pepo",
                                                bufs=2, name="pe_acc")
                            u["po"] = psum.tile([128, 512], F32, tag="pepo",
                                                bufs=2, name="po_acc")
                        he, ho = 2 * u["dt"], 2 * u["dt"] + 1
                        ep = u["ep"][st]
                        nc.tensor.matmul(u["pe"][0:65, :],
                                         vaug[:, st, he * 65:(he + 1) * 65],
                                         ep[:, 0:512],
                                         start=(st == 0), stop=(st == 15))
                        nc.tensor.matmul(u["po"][0:65, :],
                                         vaug[:, st, ho * 65:(ho + 1) * 65],
                                         ep[:, 512:1024],
                                         start=(st == 0), stop=(st == 15))
                        u["ep"][st] = None
                        u["next"] += 1
                    if u["next"] == 16:
                        unit_epilogue(u)
                        units.pop(0)
                    else:
                        break

            def normalize(cp, h, lb):
                # cp: [65, 512] f32 SBUF; row 64 = softmax denominator
                den0 = wpool.tile([1, 512], F32, tag="den0")
                nc.sync.dma_start(den0[0:1, :], cp[64:65, :])
                recb = wpool.tile([64, 512], F32, tag="recb")
                nc.gpsimd.partition_broadcast(recb[:], den0[0:1, :])
                nc.vector.reciprocal_approx_fast(recb[:], recb[:])
                dt = h // 2
                if h % 2 == 0:
                    dst = oT[0:64, dt, lb * 512:(lb + 1) * 512]
                    nc.vector.tensor_tensor(dst, cp[0:64, :], recb[:], ALU.mult)
                    nc.vector.tensor_scalar_add(dst, dst, bv_t[:, h:h + 1])
                else:
                    tmp = wpool.tile([64, 512], BF16, tag="otmp")
                    nc.vector.tensor_tensor(tmp[:], cp[0:64, :], recb[:],
                                            ALU.mult)
                    nc.vector.tensor_scalar_add(tmp[:], tmp[:], bv_t[:, h:h + 1])
                    nc.sync.dma_start(
                        oT[64:128, dt, lb * 512:(lb + 1) * 512], tmp[:])

            def unit_epilogue(u):
                dt, lb = u["dt"], u["lb"]
                cpe = wpool.tile([65, 512], F32, tag="cpe")
                nc.vector.tensor_copy(cpe[:], u["pe"][0:65, :])
                cpo = wpool.tile([65, 512], F32, tag="cpo")
                nc.vector.tensor_copy(cpo[:], u["po"][0:65, :])
                normalize(cpe, 2 * dt, lb)
                normalize(cpo, 2 * dt + 1, lb)
                norm_done[lb] += 1

            def attention_unit(dt, lb):
                ensure("q", dt, lb)
                ensure("k", dt, 0)
                u = {"dt": dt, "lb": lb, "next": 0, "exp_n": 0,
                     "ep": [None] * 16}
                units.append(u)
                qe = qT[0:64, dt, lb * 512:(lb + 1) * 512]
                qo = qT[64:128, dt, lb * 512:(lb + 1) * 512]
                for st in range(16):
                    if st % 4 == 0 and st > 0:
                        ensure("k", dt, st // 4)
                    ps2 = psum.tile([128, 1024], F32, tag="sc2", bufs=2)
                    nc.tensor.matmul(ps2[:, 0:512],
                                     kT[0:64, dt, st * 128:(st + 1) * 128],
                                     qe, start=True, stop=True)
                    nc.tensor.matmul(ps2[:, 512:1024],
                                     kT[64:128, dt, st * 128:(st + 1) * 128],
                                     qo, start=True, stop=True)
                    ep = epool.tile([128, 1024], BF16, tag="ep")
                    nc.scalar.activation(ep[:], ps2[:], ACTF.Exp, scale=0.125)
                    u["ep"][st] = ep
                    u["exp_n"] = st + 1
                    pump()
                    drain_pv()
                u["done"] = True

            # ---- startup: K(0,0)/Q(0,0) immediately, then prefetch ----
            kd, kc, _ = k_item(0, 0)
            kc(sgk0)
            qd, qc, _ = q_item(0, 0)
            qc(sgq0)
            q_box[0] = sgq0    # Q(1..3, 0) reuse the startup xq chunk

            # DMA issue order = need order: xk1 for K(0,1), wv + xv0 for the
            # first V groups, then xk2/xk3 and remaining weights
            it = k_item(0, 1, fresh=True)
            admit(it)
            emit_slot[it[2]] = 2
            nc.sync.dma_start(wv_t[:], wv[:])
            it = v_item(0, fresh=True)
            admit(it)
            emit_slot[it[2]] = 1
            it = k_item(0, 2, fresh=True)
            admit(it)
            emit_slot[it[2]] = 5
            it = k_item(0, 3, fresh=True)
            admit(it)
            emit_slot[it[2]] = 9
            nc.sync.dma_start(wq_t[:, :, 128:512], wq[:, :, 128:512])
            nc.sync.dma_start(wk_t[:, :, 128:512], wk[:, :, 128:512])
            nc.sync.dma_start(bv_t[:], bv[:])

            # ---- attention sweeps ----
            for lb in range(4):
                for dt in range(4):
                    attention_unit(dt, lb)

            # ---- flush: finish lagging PVs, then any remaining filler ----
            for _ in range(4096):
                drain_pv()
                stepped = False
                for entry in list(deferred):
                    if item_ok((None, None, entry[2])):
                        deferred.remove(entry)
                        emit(entry)
                        stepped = True
                        break
                if not stepped and pending:
                    entry = pending.pop(0)
                    if item_ok((None, None, entry[2])):
                        emit(entry)
                    else:
                        deferred.append(entry)
                    stepped = True
                if not stepped and admit_sched:
                    sl = min(admit_sched)
                    for it in admit_sched.pop(sl):
                        admit(it)
                    stepped = True
                if not stepped and not units:
                    break
            assert not units and not pending and not deferred and \
                not admit_sched

            # final O-proj groups (lt 12..15): six accumulators run their
            # ct0-2 matmuls during the last unit's PV/normalize; only ct3 +
            # epilogue (and two full groups) remain after it.
            finals = [(lt, db) for lt in range(12, 16) for db in range(2)]
            accs = []
            for i, (lt, db) in enumerate(finals[:6]):
                if i < 2:
                    ps = psum.tile([128, 512], F32, tag="mm512", bufs=2,
                                   name="oaccA")
                    ap = ps[:]
                elif i < 4:
                    ps = psum.tile([128, 1024], F32, tag="sc2", bufs=2,
                                   name="oaccB")
                    ap = ps[:, 0:512]
                else:
                    ps = psum.tile([128, 512], F32, tag="pepo", bufs=2,
                                   name="oaccC")
                    ap = ps[:]
                for ct in range(3):
                    nc.tensor.matmul(
                        ap, oT[:, ct, lt * 128:(lt + 1) * 128],
                        wo_t[:, ct, db * 512:(db + 1) * 512],
                        start=(ct == 0), stop=False)
                accs.append(ap)
            for (lt, db), ap in zip(finals[:6], accs):
                nc.tensor.matmul(
                    ap, oT[:, 3, lt * 128:(lt + 1) * 128],
                    wo_t[:, 3, db * 512:(db + 1) * 512],
                    start=False, stop=True)
                ob = wpool.tile([128, 512], F32, tag="outsb")
                nc.vector.tensor_add(ob[:], ap,
                                     bo_t[:, db * 512:(db + 1) * 512])
                nc.sync.dma_start(
                    out[lt * 128:(lt + 1) * 128,
                        db * 512:(db + 1) * 512], ob[:])
            for lt, db in finals[6:]:
                o_proj_group(lt, db)

    nc.compile()
    return nc


def _pack_kxm(w):
    k, m = w.shape
    return np.ascontiguousarray(
        w.reshape(k // 128, 128, m).transpose(1, 0, 2)).astype(ml_dtypes.bfloat16)


def kernel(queries, keys, values, Wq, bq, Wk, bk, Wv, bv, Wo, bo):
    global _nc_cache, last_results
    queries = np.asarray(queries, dtype=np.float32)
    keys = np.asarray(keys, dtype=np.float32)
    values = np.asarray(values, dtype=np.float32)

    if _nc_cache is None:
        _nc_cache = _build()
    nc = _nc_cache

    Wq = np.asarray(Wq, np.float32)
    Wk = np.asarray(Wk, np.float32)
    Wv = np.asarray(Wv, np.float32)
    Wo = np.asarray(Wo, np.float32)
    bq = np.asarray(bq, np.float32)
    bv = np.asarray(bv, np.float32)
    bo = np.asarray(bo, np.float32)

    # per head-half hh: weight slices + biases (bk dropped: softmax-invariant)
    half = {}
    for hh in range(2):
        sl = slice(hh * DH, (hh + 1) * DH)
        half[hh] = {
            "wq": _pack_kxm(Wq[:, sl]),
            "wk": _pack_kxm(Wk[:, sl]),
            "wv": _pack_kxm(Wv[:, sl]),
            "wo": _pack_kxm(Wo[sl, :]),
            "bq": np.ascontiguousarray(bq[sl].reshape(4, 128).T),
            "bv": np.ascontiguousarray(bv[sl].reshape(8, 64).T),
            "bo": np.ascontiguousarray(
                np.broadcast_to(bo, (128, D)) if hh == 0
                else np.zeros((128, D), np.float32)),
        }

    xs = {}
    for n in range(N):
        xs[n] = {
            "xq": _pack_kxm(np.ascontiguousarray(queries[n].T)),
            "xk": _pack_kxm(np.ascontiguousarray(keys[n].T)),
            "xv": _pack_kxm(np.ascontiguousarray(values[n].T)),
        }

    in_maps = []
    for c in range(N_CORES):
        n, hh = c // 2, c % 2
        m = dict(half[hh])
        m.update(xs[n])
        in_maps.append(m)

    last_results = run_bass_kernel_spmd(nc, in_maps, list(range(N_CORES)))

    full = np.empty((N, L, D), np.float32)
    for n in range(N):
        full[n] = last_results.results[2 * n]["out"]
        full[n] += last_results.results[2 * n + 1]["out"]
    return full


# revision 34
# speedup vs baseline: 1.0184x; 1.0032x over previous
"""Multi-head attention layer (N=4, L=S=2048, D=1024, H=16) on 8 TRN2 NeuronCores.

Sharding v3: 8 cores = 4 batches x 2 HEAD-HALVES (tensor parallel on heads).
Each core computes Q/K/V projections for its 8 heads (512 of 1024 dims) over
the FULL 2048 queries/keys of its batch, attention for those 8 heads, and a
PARTIAL output projection (contraction over its 512 dims). The host sums the
two partial outputs of each batch pair (unshard of the sum-sharded output).
vs the old (batch x query-half) sharding this halves the K/V projection
matmul work per core: 1536 512-col matmuls (~349us streaming) vs 1760.

Per-core data layout (host-prepared, bf16):
  xq/xk/xv [128, 8, 2048]  x[p,t,s] = input[n, s, t*128+p]   (transposed)
  wq/wk/wv [128, 8, 512]   w[p,t,d] = W[t*128+p, hh*512+d]
  wo [128, 4, 1024]        wo[p,t,d] = Wo[hh*512 + t*128+p, d]
  bq [128, 4] f32 (bk dropped: a per-query score shift is softmax-invariant)
  bv [64, 8] f32; bo [128, 1024] f32 pre-broadcast (zeros on the hh=1 core
  so the host sum adds bo exactly once)
  out [2048, 1024] f32 partial (natural layout)

Schedule: same PE-dense pending-queue design as v2 — attention units are
(dt=head-pair 0..3, lb=512-query-block 0..3); all projection groups ride as
filler inside the units; PV lags the exp stream; softmax denominator via the
ones-column of augmented V; O-projection groups gated per-lb on normalize
completion; final 8 O-groups use the early-accumulator trick so only their
last contraction step trails the last unit.
"""

import numpy as np
import ml_dtypes

import concourse.bass as bass
import concourse.mybir as mybir
import concourse.tile as tile
from concourse import bacc
from concourse.bass_utils import run_bass_kernel_spmd

BF16 = mybir.dt.bfloat16
F32 = mybir.dt.float32
ALU = mybir.AluOpType
ACTF = mybir.ActivationFunctionType

N, L, S, D, H, E = 4, 2048, 2048, 1024, 16, 64
DH = 512                 # dims per core = 8 heads
N_CORES = 8

_nc_cache = None
last_results = None


def _build():
    nc = bacc.Bacc(None, target_bir_lowering=False)

    xq = nc.declare_dram_parameter("xq", [128, 8, L], BF16, isOutput=False)
    xk = nc.declare_dram_parameter("xk", [128, 8, S], BF16, isOutput=False)
    xv = nc.declare_dram_parameter("xv", [128, 8, S], BF16, isOutput=False)
    wq = nc.declare_dram_parameter("wq", [128, 8, DH], BF16, isOutput=False)
    wk = nc.declare_dram_parameter("wk", [128, 8, DH], BF16, isOutput=False)
    wv = nc.declare_dram_parameter("wv", [128, 8, DH], BF16, isOutput=False)
    wo = nc.declare_dram_parameter("wo", [128, 4, D], BF16, isOutput=False)
    bq = nc.declare_dram_parameter("bq", [128, 4], F32, isOutput=False)
    bv = nc.declare_dram_parameter("bv", [64, 8], F32, isOutput=False)
    bo = nc.declare_dram_parameter("bo", [128, D], F32, isOutput=False)
    out = nc.declare_dram_parameter("out", [L, D], F32, isOutput=True)

    with tile.TileContext(nc) as tc:
        with tc.tile_pool(name="const", bufs=1) as cpool, \
             tc.tile_pool(name="pers", bufs=1) as ppool, \
             tc.tile_pool(name="stage", bufs=3) as spool, \
             tc.tile_pool(name="vstage", bufs=2) as vpool, \
             tc.tile_pool(name="qstage", bufs=2) as qpool, \
             tc.tile_pool(name="work", bufs=2) as wpool, \
             tc.tile_pool(name="expp", bufs=7) as epool, \
             tc.tile_pool(name="psum", bufs=2, space="PSUM") as psum:

            wq_t = cpool.tile([128, 8, DH], BF16, tag="w_q")
            wk_t = cpool.tile([128, 8, DH], BF16, tag="w_k")
            wv_t = cpool.tile([128, 8, DH], BF16, tag="w_v")
            wo_t = cpool.tile([128, 4, D], BF16, tag="w_o")
            bq_t = cpool.tile([128, 4], F32, tag="bq")
            bv_t = cpool.tile([64, 8], F32, tag="bv")
            bo_t = cpool.tile([128, D], F32, tag="bo")
            qT = ppool.tile([128, 4, L], BF16, tag="qT")
            kT = ppool.tile([128, 4, S], BF16, tag="kT")
            vaug = ppool.tile([128, 16, 8 * 65], BF16, tag="vaug")
            oT = ppool.tile([128, 4, L], BF16, tag="oT")

            # ---- critical-path DMAs: K(0,0) + Q(0,0) inputs first ----
            # ct0 slivers first so the first matmul can start ~1us in
            nc.sync.dma_start(wk_t[:, 0:1, 0:128], wk[:, 0:1, 0:128])
            sgk0 = spool.tile([128, 8, 512], BF16, tag="stage")
            nc.sync.dma_start(sgk0[:, 0:1, :], xk[:, 0:1, 0:512])
            nc.sync.dma_start(wk_t[:, 1:8, 0:128], wk[:, 1:8, 0:128])
            nc.sync.dma_start(sgk0[:, 1:8, :], xk[:, 1:8, 0:512])
            nc.sync.dma_start(wq_t[:, :, 0:128], wq[:, :, 0:128])
            sgq0 = qpool.tile([128, 8, 512], BF16, tag="qstage")
            nc.sync.dma_start(sgq0[:], xq[:, :, 0:512])
            nc.sync.dma_start(bq_t[:], bq[:])

            # warm the exp table-set while DMAs fill (one tiny ACTIVATE)
            wrm = wpool.tile([1, 16], F32, tag="warm")
            nc.vector.memset(wrm[:], 0.0)
            wrm2 = wpool.tile([1, 16], F32, tag="warm2")
            nc.scalar.activation(wrm2[:], wrm[:], ACTF.Exp, scale=0.125)

            # ones column (slot 64) of augmented V => PV row 64 = softmax denom
            for st in range(16):
                v3 = vaug[:, st].rearrange("p (h e) -> p h e", e=65)
                nc.vector.memset(v3[:, :, 64:65], 1.0)

            # ---- projection-group emitters ----
            def proj_group(w_t, sg_t, dt, dst, bias, on_act=False):
                ps = psum.tile([128, 512], F32, tag="mm512", bufs=2)
                for ct in range(8):
                    nc.tensor.matmul(ps[:], w_t[:, ct, dt * 128:(dt + 1) * 128],
                                     sg_t[:, ct, :], start=(ct == 0),
                                     stop=(ct == 7))
                # evacuate on the scalar engine while it still has idle time
                # (era0) so the DVE FIFO doesn't delay mm512 psum recycling
                if on_act:
                    nc.scalar.activation(dst, ps[:], ACTF.Identity,
                                         bias=0.0 if bias is None else bias)
                elif bias is None:
                    nc.vector.tensor_copy(dst, ps[:])
                else:
                    nc.vector.tensor_scalar_add(dst, ps[:], bias)

            def v_proj_group(sg_t, st):
                stl = st % 4
                ps = psum.tile([128, 512], F32, tag="mm512", bufs=2)
                for ct in range(8):
                    nc.tensor.matmul(ps[:], sg_t[:, ct, stl * 128:(stl + 1) * 128],
                                     wv_t[:, ct, :], start=(ct == 0),
                                     stop=(ct == 7))
                v3 = vaug[:, st].rearrange("p (h e) -> p h e", e=65)
                # evacuate on the scalar engine (idle during era0) so the DVE
                # FIFO doesn't delay mm512 psum recycling for later groups
                nc.scalar.activation(
                    v3[:, :, 0:64],
                    ps[:].rearrange("p (h e) -> p h e", e=64),
                    ACTF.Identity)

            def o_proj_group(lt, db):
                ps = psum.tile([128, 512], F32, tag="mm512", bufs=2)
                for ct in range(4):
                    nc.tensor.matmul(ps[:], oT[:, ct, lt * 128:(lt + 1) * 128],
                                     wo_t[:, ct, db * 512:(db + 1) * 512],
                                     start=(ct == 0), stop=(ct == 3))
                ob = wpool.tile([128, 512], F32, tag="outsb")
                nc.vector.tensor_add(ob[:], ps[:],
                                     bo_t[:, db * 512:(db + 1) * 512])
                nc.sync.dma_start(
                    out[lt * 128:(lt + 1) * 128, db * 512:(db + 1) * 512], ob[:])

            # ---- feed machinery ----
            emitted = set()        # keys of emitted groups
            v_ready = [0]          # count of emitted V st-groups
            norm_done = [0, 0, 0, 0]   # per lb: fully-normalized units

            k_box, q_box, v_box = [None], [None], [None]

            def k_item(dt, sb, fresh=False):
                def dma():
                    if fresh:
                        sg = spool.tile([128, 8, 512], BF16, tag="stage")
                        nc.sync.dma_start(sg[:],
                                          xk[:, :, sb * 512:(sb + 1) * 512])
                        k_box[0] = sg
                    return k_box[0]
                def compute(sg):
                    proj_group(wk_t, sg, dt, kT[:, dt, sb * 512:(sb + 1) * 512],
                               None)
                    emitted.add(("k", dt, sb))
                return (dma, compute, ("k", dt, sb))

            def q_item(dt, lb, fresh=False):
                def dma():
                    if fresh:
                        sg = qpool.tile([128, 8, 512], BF16, tag="qstage")
                        nc.sync.dma_start(sg[:],
                                          xq[:, :, lb * 512:(lb + 1) * 512])
                        q_box[0] = sg
                    return q_box[0]
                def compute(sg):
                    proj_group(wq_t, sg, dt, qT[:, dt, lb * 512:(lb + 1) * 512],
                               bq_t[:, dt:dt + 1])
                    emitted.add(("q", dt, lb))
                return (dma, compute, ("q", dt, lb))

            def v_item(st, fresh=False):
                sb = st // 4
                def dma():
                    if fresh:
                        sg = vpool.tile([128, 8, 512], BF16, tag="vstage")
                        nc.sync.dma_start(sg[:],
                                          xv[:, :, sb * 512:(sb + 1) * 512])
                        v_box[0] = sg
                    return v_box[0]
                def compute(sg):
                    v_proj_group(sg, st)
                    v_ready[0] += 1
                    emitted.add(("v", st, 0))
                return (dma, compute, ("v", st, 0))

            def dma_feed(fn, name):
                return (None, lambda sg: fn(), ("w", name, 0))

            def o_item(lt, db):
                def compute(sg):
                    o_proj_group(lt, db)
                    emitted.add(("o", lt, db))
                return (None, compute, ("o", lt, db))

            # ---- slot-scheduled feed: emit each filler group near its
            # need-slot so PE filler spreads across all 256 unit st-slots
            # (a clustered feed leaves later eras exp-latency-bound).
            # admit (=issue input DMA) LEAD slots before the emit slot.
            LEAD = 5
            admit_sched = {}       # slot -> [item]
            emit_slot = {}         # key -> scheduled emit slot
            slot_ctr = [0]
            pending = []           # [(compute, sg, key)] admitted, not emitted
            deferred = []          # admitted but gated at emit time

            def at(sl, it):
                admit_sched.setdefault(max(0, sl - LEAD), []).append(it)
                emit_slot[it[2]] = sl

            def item_ok(it):
                key = it[2]
                return key[0] != "o" or norm_done[key[1] // 4] >= 4

            def emit(entry):
                compute, sg, key = entry
                compute(sg)

            def admit(it):
                dma, compute, key = it
                sg = dma() if dma else None
                pending.append((compute, sg, key))

            def pump():
                sl = slot_ctr[0]
                slot_ctr[0] += 1
                for it in admit_sched.pop(sl, []):
                    admit(it)
                for entry in list(deferred):
                    if item_ok((None, None, entry[2])):
                        deferred.remove(entry)
                        emit(entry)
                while pending and emit_slot.get(pending[0][2], 0) <= sl:
                    entry = pending.pop(0)
                    if item_ok((None, None, entry[2])):
                        emit(entry)
                    else:
                        deferred.append(entry)

            def ensure(kind, a, b):
                key = (kind, a, b)
                guard = 0
                while key not in emitted:
                    guard += 1
                    if guard > 10000:
                        raise RuntimeError(f"cannot satisfy prereq {key}")
                    # force-drain: admit everything scheduled so far, emit
                    # pending in order until the prereq lands
                    if pending:
                        emit(pending.pop(0))
                    elif admit_sched:
                        sl = min(admit_sched)
                        for it in admit_sched.pop(sl):
                            admit(it)
                    else:
                        raise RuntimeError(f"cannot satisfy prereq {key}")

            # schedule: V(st) ~st; K(dt,sb) before unit(dt,0) consumes it;
            # Q(dt,lb) ~8 slots before unit(dt,lb); O(lb) spread over the 64
            # slots after lb completes; wo/bo mid-era0.
            for st in range(1, 16):
                at(st, v_item(st, fresh=(st % 4 == 0)))
            for dt in range(1, 4):
                for sb in range(4):
                    at(16 * dt + 4 * sb - 6, k_item(dt, sb, fresh=True))
            for lb in range(4):
                for dt in range(4):
                    if lb == 0 and dt == 0:
                        continue
                    sl = 64 * lb + 16 * dt - 8
                    at(sl, q_item(dt, lb, fresh=(lb > 0 and dt == 0)))
            at(40, dma_feed(lambda: nc.sync.dma_start(wo_t[:], wo[:]), "wo"))
            at(44, dma_feed(lambda: nc.sync.dma_start(bo_t[:], bo[:]), "bo"))
            for lb in range(3):
                for i, (lt, db) in enumerate(
                        (lt, db) for lt in range(4 * lb, 4 * lb + 4)
                        for db in range(2)):
                    at(64 * (lb + 1) + 2 + 7 * i, o_item(lt, db))

            # ---- attention units with lagging PV ----
            units = []

            def drain_pv():
                while units:
                    u = units[0]
                    # keep PV two steps behind the exp stream while the
                    # unit's scores are still running, so PV matmuls never
                    # reach the engine before their exp has completed
                    lim = u["exp_n"] if u.get("done") else u["exp_n"] - 2
                    while (u["next"] < 16 and u["next"] < v_ready[0]
                           and u["next"] < lim):
                        st = u["next"]
                        if st == 0:
                            u["pe"] = psum.tile([128, 512], F32, tag="pepo",
                                                bufs=2, name="pe_acc")
                            u["po"] = psum.tile([128, 512], F32, tag="pepo",
                                                bufs=2, name="po_acc")
                        he, ho = 2 * u["dt"], 2 * u["dt"] + 1
                        ep = u["ep"][st]
                        nc.tensor.matmul(u["pe"][0:65, :],
                                         vaug[:, st, he * 65:(he + 1) * 65],
                                         ep[:, 0:512],
                                         start=(st == 0), stop=(st == 15))
                        nc.tensor.matmul(u["po"][0:65, :],
                                         vaug[:, st, ho * 65:(ho + 1) * 65],
                                         ep[:, 512:1024],
                                         start=(st == 0), stop=(st == 15))
                        u["ep"][st] = None
                        u["next"] += 1
                    if u["next"] == 16:
                        unit_epilogue(u)
                        units.pop(0)
                    else:
                        break

            def normalize(cp, h, lb):
                # cp: [65, 512] f32 SBUF; row 64 = softmax denominator
                den0 = wpool.tile([1, 512], F32, tag="den0")
                nc.sync.dma_start(den0[0:1, :], cp[64:65, :])
                recb = wpool.tile([64, 512], F32, tag="recb")
                nc.gpsimd.partition_broadcast(recb[:], den0[0:1, :])
                nc.vector.reciprocal_approx_fast(recb[:], recb[:])
                dt = h // 2
                if h % 2 == 0:
                    dst = oT[0:64, dt, lb * 512:(lb + 1) * 512]
                    nc.vector.tensor_tensor(dst, cp[0:64, :], recb[:], ALU.mult)
                    nc.vector.tensor_scalar_add(dst, dst, bv_t[:, h:h + 1])
                else:
                    tmp = wpool.tile([64, 512], BF16, tag="otmp")
                    nc.vector.tensor_tensor(tmp[:], cp[0:64, :], recb[:],
                                            ALU.mult)
                    nc.vector.tensor_scalar_add(tmp[:], tmp[:], bv_t[:, h:h + 1])
                    nc.sync.dma_start(
                        oT[64:128, dt, lb * 512:(lb + 1) * 512], tmp[:])

            def unit_epilogue(u):
                dt, lb = u["dt"], u["lb"]
                cpe = wpool.tile([65, 512], F32, tag="cpe")
                nc.vector.tensor_copy(cpe[:], u["pe"][0:65, :])
                cpo = wpool.tile([65, 512], F32, tag="cpo")
                nc.vector.tensor_copy(cpo[:], u["po"][0:65, :])
                normalize(cpe, 2 * dt, lb)
                normalize(cpo, 2 * dt + 1, lb)
                norm_done[lb] += 1

            def attention_unit(dt, lb):
                ensure("q", dt, lb)
                ensure("k", dt, 0)
                u = {"dt": dt, "lb": lb, "next": 0, "exp_n": 0,
                     "ep": [None] * 16}
                units.append(u)
                qe = qT[0:64, dt, lb * 512:(lb + 1) * 512]
                qo = qT[64:128, dt, lb * 512:(lb + 1) * 512]
                for st in range(16):
                    if st % 4 == 0 and st > 0:
                        ensure("k", dt, st // 4)
                    ps2 = psum.tile([128, 1024], F32, tag="sc2", bufs=2)
                    nc.tensor.matmul(ps2[:, 0:512],
                                     kT[0:64, dt, st * 128:(st + 1) * 128],
                                     qe, start=True, stop=True)
                    nc.tensor.matmul(ps2[:, 512:1024],
                                     kT[64:128, dt, st * 128:(st + 1) * 128],
                                     qo, start=True, stop=True)
                    ep = epool.tile([128, 1024], BF16, tag="ep")
                    nc.scalar.activation(ep[:], ps2[:], ACTF.Exp, scale=0.125)
                    u["ep"][st] = ep
                    u["exp_n"] = st + 1
                    pump()
                    drain_pv()
                u["done"] = True

            # ---- startup: K(0,0)/Q(0,0) immediately, then prefetch ----
            kd, kc, _ = k_item(0, 0)
            kc(sgk0)
            qd, qc, _ = q_item(0, 0)
            qc(sgq0)
            q_box[0] = sgq0    # Q(1..3, 0) reuse the startup xq chunk

            # DMA issue order = need order: xk1 for K(0,1), wv + xv0 for the
            # first V groups, then xk2/xk3 and remaining weights
            it = k_item(0, 1, fresh=True)
            admit(it)
            emit_slot[it[2]] = 2
            nc.sync.dma_start(wv_t[:], wv[:])
            it = v_item(0, fresh=True)
            admit(it)
            emit_slot[it[2]] = 1
            it = k_item(0, 2, fresh=True)
            admit(it)
            emit_slot[it[2]] = 5
            it = k_item(0, 3, fresh=True)
            admit(it)
            emit_slot[it[2]] = 9
            nc.sync.dma_start(wq_t[:, :, 128:512], wq[:, :, 128:512])
            nc.sync.dma_start(wk_t[:, :, 128:512], wk[:, :, 128:512])
            nc.sync.dma_start(bv_t[:], bv[:])

            # ---- attention sweeps ----
            for lb in range(4):
                for dt in range(4):
                    attention_unit(dt, lb)

            # ---- flush: finish lagging PVs, then any remaining filler ----
            for _ in range(4096):
                drain_pv()
                stepped = False
                for entry in list(deferred):
                    if item_ok((None, None, entry[2])):
                        deferred.remove(entry)
                        emit(entry)
                        stepped = True
                        break
                if not stepped and pending:
                    entry = pending.pop(0)
                    if item_ok((None, None, entry[2])):
                        emit(entry)
                    else:
                        deferred.append(entry)
                    stepped = True
                if not stepped and admit_sched:
                    sl = min(admit_sched)
                    for it in admit_sched.pop(sl):
                        admit(it)
                    stepped = True
                if not stepped and not units:
                    break
            assert not units and not pending and not deferred and \
                not admit_sched

            # final O-proj groups (lt 12..15): six accumulators run their
            # ct0-2 matmuls during the last unit's PV/normalize; only ct3 +
            # epilogue (and two full groups) remain after it.
            finals = [(lt, db) for lt in range(12, 16) for db in range(2)]
            accs = []
            for i, (lt, db) in enumerate(finals[:6]):
                if i < 2:
                    ps = psum.tile([128, 512], F32, tag="mm512", bufs=2,
                                   name="oaccA")
                    ap = ps[:]
                elif i < 4:
                    ps = psum.tile([128, 1024], F32, tag="sc2", bufs=2,
                                   name="oaccB")
                    ap = ps[:, 0:512]
                else:
                    ps = psum.tile([128, 512], F32, tag="pepo", bufs=2,
                                   name="oaccC")
                    ap = ps[:]
                for ct in range(3):
                    nc.tensor.matmul(
                        ap, oT[:, ct, lt * 128:(lt + 1) * 128],
                        wo_t[:, ct, db * 512:(db + 1) * 512],
                        start=(ct == 0), stop=False)
                accs.append(ap)
            for (lt, db), ap in zip(finals[:6], accs):
                nc.tensor.matmul(
                    ap, oT[:, 3, lt * 128:(lt + 1) * 128],
                    wo_t[:, 3, db * 512:(db + 1) * 512],
                    start=False, stop=True)
                ob = wpool.tile([128, 512], F32, tag="outsb")
                nc.vector.tensor_add(ob[:], ap,
                                     bo_t[:, db * 512:(db + 1) * 512])
                nc.sync.dma_start(
                    out[lt * 128:(lt + 1) * 128,
                        db * 512:(db + 1) * 512], ob[:])
            for lt, db in finals[6:]:
                o_proj_group(lt, db)

    nc.compile()
    return nc


def _pack_kxm(w):
    k, m = w.shape
    return np.ascontiguousarray(
        w.reshape(k // 128, 128, m).transpose(1, 0, 2)).astype(ml_dtypes.bfloat16)


def kernel(queries, keys, values, Wq, bq, Wk, bk, Wv, bv, Wo, bo):
    global _nc_cache, last_results
    queries = np.asarray(queries, dtype=np.float32)
    keys = np.asarray(keys, dtype=np.float32)
    values = np.asarray(values, dtype=np.float32)

    if _nc_cache is None:
        _nc_cache = _build()
    nc = _nc_cache

    Wq = np.asarray(Wq, np.float32)
    Wk = np.asarray(Wk, np.float32)
    Wv = np.asarray(Wv, np.float32)
    Wo = np.asarray(Wo, np.float32)
    bq = np.asarray(bq, np.float32)
    bv = np.asarray(bv, np.float32)
    bo = np.asarray(bo, np.float32)

    # per head-half hh: weight slices + biases (bk dropped: softmax-invariant)
    half = {}
    for hh in range(2):
        sl = slice(hh * DH, (hh + 1) * DH)
        half[hh] = {
            "wq": _pack_kxm(Wq[:, sl]),
            "wk": _pack_kxm(Wk[:, sl]),
            "wv": _pack_kxm(Wv[:, sl]),
            "wo": _pack_kxm(Wo[sl, :]),
            "bq": np.ascontiguousarray(bq[sl].reshape(4, 128).T),
            "bv": np.ascontiguousarray(bv[sl].reshape(8, 64).T),
            "bo": np.ascontiguousarray(
                np.broadcast_to(bo, (128, D)) if hh == 0
                else np.zeros((128, D), np.float32)),
        }

    xs = {}
    for n in range(N):
        xs[n] = {
            "xq": _pack_kxm(np.ascontiguousarray(queries[n].T)),
            "xk": _pack_kxm(np.ascontiguousarray(keys[n].T)),
            "xv": _pack_kxm(np.ascontiguousarray(values[n].T)),
        }

    in_maps = []
    for c in range(N_CORES):
        n, hh = c // 2, c % 2
        m = dict(half[hh])
        m.update(xs[n])
        in_maps.append(m)

    last_results = run_bass_kernel_spmd(nc, in_maps, list(range(N_CORES)))

    full = np.empty((N, L, D), np.float32)
    for n in range(N):
        full[n] = last_results.results[2 * n]["out"]
        full[n] += last_results.results[2 * n + 1]["out"]
    return full
